# revision 6
# baseline (speedup 1.0000x reference)
"""Bahdanau attention Trainium2 kernel (8 NeuronCores, data-parallel on batch).

Reference computation (B=32, T=512, DH=2048, fp32):
    query = dec @ Ws_w.T + Ws_b          (t == 0)
    q     = query @ Wa_w.T + Wa_b                     # [B, DH]   (tiny -> host)
    pk    = keys @ Ua_w.T + Ua_b                      # [B, T, DH] (the 137 GFLOP matmul)
    e     = tanh(q[:, None, :] + pk)
    scores= e . Va                                    # [B, T]  (+Va_b: softmax-invariant, dropped)
    attn  = softmax(scores, axis=1)
    av    = attn @ keys                               # [B, DH]

Device strategy per core (4 batches/core):
    - keys^T [DH, 4*T] and Ua^T [DH, DH] fed transposed+bf16 from host
      (contraction dim on partitions; PE needs both operands K-major).
    - main matmul: out[e_chunk(128), t(512)] accumulating 16 k-chunks in PSUM.
    - ACT: tanh(psum + bias) fused, bias = (q+Ua_b)^T per-partition column.
    - scores: PE matmuls lhsT=Va chunk [128,1] accumulated over 16 e-chunks.
    - softmax on [1, 512]: DVE max(negate) -> ACT exp(bias=-max, accum=sum)
      -> DVE reciprocal -> DVE scale.
    - replicate attn to 128 partitions via PE (ones[1,128]^T @ attn[1,512]).
    - av: fused DVE tensor_tensor_reduce over keysT tiles -> av^T [128,16],
      PE-transpose -> [16,128] -> DMA out.
"""

import numpy as np
import ml_dtypes

import concourse.bacc as bacc
import concourse.mybir as mybir
import concourse.tile as tile
from concourse.masks import make_identity
from concourse.bass_utils import run_bass_kernel_spmd

B, T, DH = 32, 512, 2048
NCORES = 8
BPC = B // NCORES          # batches per core
P = 128
KC = DH // P               # contraction chunks
EC = DH // P               # output-feature chunks

F32 = mybir.dt.float32
BF16 = mybir.dt.bfloat16
MM_DT = BF16               # dtype of the big matmuls (bf16: full PE rate)

_nbf = ml_dtypes.bfloat16


def build_program(loop_n=None):
    """Build the per-core Bass program. loop_n wraps the body in a hardware
    loop (timing harness only); None = single-shot (grading path)."""
    nc = bacc.Bacc("TRN2", target_bir_lowering=False, debug=False,
                   num_devices=NCORES)

    keysT = nc.dram_tensor("keysT", [DH, BPC * T], MM_DT, kind="ExternalInput")
    uaT = nc.dram_tensor("uaT", [DH, DH], MM_DT, kind="ExternalInput")
    qbT = nc.dram_tensor("qbT", [P, EC * BPC], F32, kind="ExternalInput")
    vaT = nc.dram_tensor("vaT", [P, EC], MM_DT, kind="ExternalInput")
    av_out = nc.dram_tensor("av_out", [BPC, DH], F32, kind="ExternalOutput")
    attn_out = nc.dram_tensor("attn_out", [BPC, T], F32, kind="ExternalOutput")

    av_dram = av_out.ap().rearrange("b (c p) -> b c p", c=EC)  # [BPC, 16, 128]

    with tile.TileContext(nc) as tc:
        with (
            tc.tile_pool(name="const", bufs=1) as const_pool,
            tc.tile_pool(name="ua", bufs=KC) as ua_pool,
            tc.tile_pool(name="kt", bufs=2 * KC) as kt_pool,
            tc.tile_pool(name="th", bufs=KC + 4) as th_pool,
            tc.tile_pool(name="small", bufs=8) as small_pool,
            tc.tile_pool(name="scr", bufs=2) as scr_pool,
            tc.tile_pool(name="ps_pk", bufs=4, space="PSUM") as ps_pk_pool,
            tc.tile_pool(name="ps_sc", bufs=1, space="PSUM") as ps_sc_pool,
            tc.tile_pool(name="ps_rep", bufs=1, space="PSUM") as ps_rep_pool,
            tc.tile_pool(name="ps_av", bufs=1, space="PSUM") as ps_av_pool,
        ):
            identity = const_pool.tile([P, P], F32)
            make_identity(nc, identity[:, :])
            ones = const_pool.tile([1, P], F32)
            nc.vector.memset(ones[:, :], 1.0)

            def body():
                # static operands (re-fetched per iteration in loop mode; the
                # DMA overlaps compute either way)
                ua_sb = []
                for k in range(KC):
                    t_ = ua_pool.tile([P, DH], MM_DT, tag="ua")
                    nc.sync.dma_start(t_[:, :], uaT.ap()[k * P:(k + 1) * P, :])
                    ua_sb.append(t_)
                qb_sb = small_pool.tile([P, EC * BPC], F32, tag="qb")
                nc.sync.dma_start(qb_sb[:, :], qbT.ap())
                va_sb = small_pool.tile([P, EC], MM_DT, tag="va")
                nc.sync.dma_start(va_sb[:, :], vaT.ap())

                for b in range(BPC):
                    # keys^T tiles for this batch: [128(d), 512(t)] x KC
                    kt = []
                    for k in range(KC):
                        t_ = kt_pool.tile([P, T], MM_DT, tag="kt")
                        nc.sync.dma_start(
                            t_[:, :],
                            keysT.ap()[k * P:(k + 1) * P, b * T:(b + 1) * T])
                        kt.append(t_)

                    # pk[e] = sum_k UaT[k,e]^T @ keysT[k]  -> tanh(+bias)
                    th = []
                    for e in range(EC):
                        ps = ps_pk_pool.tile([P, T], F32, tag="ps_pk")
                        for k in range(KC):
                            nc.tensor.matmul(
                                ps[:, :],
                                ua_sb[k][:, e * P:(e + 1) * P],
                                kt[k][:, :],
                                start=(k == 0), stop=(k == KC - 1))
                        t_ = th_pool.tile([P, T], MM_DT, tag="th")
                        nc.scalar.activation(
                            t_[:, :], ps[:, :],
                            mybir.ActivationFunctionType.Tanh,
                            bias=qb_sb[:, e * BPC + b:e * BPC + b + 1],
                            scale=1.0)
                        th.append(t_)

                    # scores[1, 512] = sum_e Va_chunk[e]^T @ th[e]
                    ps_sc = ps_sc_pool.tile([1, T], F32, tag="ps_sc")
                    for e in range(EC):
                        nc.tensor.matmul(
                            ps_sc[:, :], va_sb[:, e:e + 1], th[e][:, :],
                            start=(e == 0), stop=(e == EC - 1))

                    # softmax over the 512 scores (single partition)
                    negmax = small_pool.tile([1, 1], F32, tag="negmax")
                    nc.vector.tensor_reduce(
                        negmax[:, :], ps_sc[:, :],
                        axis=mybir.AxisListType.X, op=mybir.AluOpType.max,
                        negate=True)
                    exp_sb = small_pool.tile([1, T], F32, tag="exp")
                    sumexp = small_pool.tile([1, 1], F32, tag="sumexp")
                    nc.scalar.activation(
                        exp_sb[:, :], ps_sc[:, :],
                        mybir.ActivationFunctionType.Exp,
                        bias=negmax[:, :], scale=1.0,
                        accum_out=sumexp[:, :])
                    inv = small_pool.tile([1, 1], F32, tag="inv")
                    nc.vector.reciprocal(inv[:, :], sumexp[:, :])
                    attn_sb = small_pool.tile([1, T], F32, tag="attn")
                    nc.vector.tensor_scalar_mul(
                        attn_sb[:, :], exp_sb[:, :], inv[:, :])
                    nc.sync.dma_start(attn_out.ap()[b:b + 1, :], attn_sb[:, :])

                    # replicate attn across partitions: ones^T @ attn
                    ps_rep = ps_rep_pool.tile([P, T], F32, tag="ps_rep")
                    nc.tensor.matmul(ps_rep[:, :], ones[:, :], attn_sb[:, :],
                                     start=True, stop=True)

                    # av^T[:, d] = sum_t keysT[d][:, t] * attn[t]
                    # (tensor_tensor_reduce is broken in this backend --
                    #  use separate multiply + reduce)
                    av_t = small_pool.tile([P, EC], F32, tag="av_t")
                    for d in range(KC):
                        scr = scr_pool.tile([P, T], F32, tag="scr")
                        nc.vector.tensor_tensor(
                            out=scr[:, :], in0=kt[d][:, :], in1=ps_rep[:, :],
                            op=mybir.AluOpType.mult)
                        nc.vector.reduce_sum(
                            av_t[:, d:d + 1], scr[:, :],
                            axis=mybir.AxisListType.X)

                    # transpose [128,16] -> [16,128], copy out
                    ps_av = ps_av_pool.tile([EC, P], F32, tag="ps_av")
                    nc.tensor.transpose(ps_av[:, :], av_t[:, :],
                                        identity[:, :])
                    av_sb = small_pool.tile([EC, P], F32, tag="av_sb")
                    nc.scalar.copy(av_sb[:, :], ps_av[:, :])
                    nc.sync.dma_start(av_dram[b], av_sb[:, :])

            if loop_n is None:
                body()
            else:
                with tc.For_i(0, loop_n, 1,
                              hint_engines=(mybir.EngineType.PE,)):
                    body()

    nc.compile()
    return nc


_CACHE = {}


def _get_program(loop_n=None):
    key = ("prog", loop_n)
    if key not in _CACHE:
        _CACHE[key] = build_program(loop_n)
    return _CACHE[key]


def _make_runner(nc):
    """Build a cached jitted SPMD executor for `nc` (the library's
    run_bass_kernel_spmd re-traces jax.jit on every call, which costs ~3s;
    tracing once makes repeat calls cheap)."""
    import jax
    from jax.experimental.shard_map import shard_map
    from jax.sharding import Mesh, PartitionSpec
    from concourse.bass2jax import (_bass_exec_p, install_neuronx_cc_hook,
                                    partition_id_tensor)

    install_neuronx_cc_hook()

    partition_name = (nc.partition_id_tensor.name
                      if nc.partition_id_tensor else None)
    in_names, out_names, out_avals, zero_shapes = [], [], [], []
    for alloc in nc.m.functions[0].allocations:
        if not isinstance(alloc, mybir.MemoryLocationSet):
            continue
        name = alloc.memorylocations[0].name
        if alloc.kind == "ExternalInput":
            if name != partition_name:
                in_names.append(name)
        elif alloc.kind == "ExternalOutput":
            shape = tuple(alloc.tensor_shape)
            dtype = mybir.dt.np(alloc.dtype)
            out_names.append(name)
            out_avals.append(jax.core.ShapedArray(shape, dtype))
            zero_shapes.append((shape, dtype))
    n_params = len(in_names)
    all_names = list(in_names + out_names)
    if partition_name is not None:
        all_names.append(partition_name)
    all_names = tuple(all_names)
    donate = tuple(range(n_params, n_params + len(out_names)))

    def _body(*args):
        operands = list(args)
        if partition_name is not None:
            operands.append(partition_id_tensor())
        outs = _bass_exec_p.bind(
            *operands,
            out_avals=tuple(out_avals),
            in_names=all_names,
            out_names=tuple(out_names),
            lowering_input_output_aliases=(),
            sim_require_finite=True,
            sim_require_nnan=True,
            nc=nc,
        )
        return tuple(outs)

    devices = jax.devices()[:NCORES]
    mesh = Mesh(np.asarray(devices), ("core",))
    nio = n_params + len(out_names)
    sharded = jax.jit(
        shard_map(_body, mesh=mesh, in_specs=(PartitionSpec("core"),) * nio,
                  out_specs=(PartitionSpec("core"),) * len(out_names),
                  check_rep=False),
        donate_argnums=donate, keep_unused=True)

    def run(in_maps):
        concat_in = [
            np.concatenate([np.asarray(m[name]) for m in in_maps], axis=0)
            for name in in_names
        ]
        concat_zeros = [
            np.zeros((NCORES * s[0], *s[1:]), d) for s, d in zero_shapes
        ]
        out_arrs = sharded(*concat_in, *concat_zeros)
        return [
            {name: np.asarray(out_arrs[i]).reshape(
                NCORES, *out_avals[i].shape)[c]
             for i, name in enumerate(out_names)}
            for c in range(NCORES)
        ]

    run.sharded = sharded
    run.in_names = in_names
    run.out_names = out_names
    run.zero_shapes = zero_shapes
    run.out_avals = out_avals
    run.mesh = mesh
    return run


def _get_runner(loop_n=None):
    key = ("runner", loop_n)
    if key not in _CACHE:
        _CACHE[key] = _make_runner(_get_program(loop_n))
    return _CACHE[key]


def _prep_inputs(inputs):
    keys = np.asarray(inputs["keys"], dtype=np.float32)
    dec = np.asarray(inputs["decoder_state"], dtype=np.float32)
    Ws_w = np.asarray(inputs["Ws_w"], dtype=np.float32)
    Ws_b = np.asarray(inputs["Ws_b"], dtype=np.float32)
    Wa_w = np.asarray(inputs["Wa_w"], dtype=np.float32)
    Wa_b = np.asarray(inputs["Wa_b"], dtype=np.float32)
    Ua_w = np.asarray(inputs["Ua_w"], dtype=np.float32)
    Ua_b = np.asarray(inputs["Ua_b"], dtype=np.float32)
    Va_w = np.asarray(inputs["Va_w"], dtype=np.float32)
    t = int(np.asarray(inputs["t"]))

    query = dec @ Ws_w.T + Ws_b if t == 0 else dec
    q = query @ Wa_w.T + Wa_b                      # [B, DH]
    qb = q + Ua_b[None, :]                         # fold Ua bias into ACT bias

    uaT = np.ascontiguousarray(Ua_w.T).astype(_nbf)          # [DH(d), DH(e)]
    vaT = np.ascontiguousarray(
        Va_w[0].reshape(EC, P).T).astype(_nbf)               # [128, 16]

    keys_bf = keys.astype(_nbf)
    in_maps = []
    for c in range(NCORES):
        bs = slice(c * BPC, (c + 1) * BPC)
        keysT_c = np.ascontiguousarray(
            keys_bf[bs].reshape(BPC * T, DH).T)              # [DH, BPC*T]
        # qbT[p, e*BPC + b] = qb[b, e*128 + p]
        qbT_c = np.ascontiguousarray(
            qb[bs].reshape(BPC, EC, P).transpose(2, 1, 0).reshape(P, EC * BPC))
        in_maps.append({
            "keysT": keysT_c,
            "uaT": uaT,
            "qbT": qbT_c.astype(np.float32),
            "vaT": vaT,
        })
    return in_maps


def _assemble(results):
    av = np.concatenate([results[c]["av_out"] for c in range(NCORES)], axis=0)
    attn = np.concatenate([results[c]["attn_out"] for c in range(NCORES)],
                          axis=0)
    return av.astype(np.float32), attn[:, :, None].astype(np.float32)


def kernel(**inputs):
    in_maps = _prep_inputs(inputs)
    run = _get_runner()
    return _assemble(run(in_maps))
